# revision 17
# baseline (speedup 1.0000x reference)
"""Trainium2 Bass kernel for RSVFiLM (moe_routing).

Math (per batch b):
  Z_up = bilinear2x(Z[b])  [64, 80, 80]
  P_up = bilinear2x(P[b])  [3, 80, 80]
  u[j, n] rows: j in [0..195]:
     rows 0..63   : Z_up[d] * P_up[0]
     rows 64..127 : Z_up[d] * P_up[1]
     rows 128..191: Z_up[d] * P_up[2]
     rows 192..194: P_up[k]
     row 195      : 1.0
  [gamma_total; delta_beta] = U.T @ u   (U [196, 512] combines Wg/bg/+1 and Wb/bb)
  out = feat * gamma_total + delta_beta

Sharding: pure data-parallel, 2 batches per core across 8 cores. Expert
weights replicated. All device math in bf16 (fp32 PSUM accumulation).
"""

import numpy as np
import ml_dtypes

B, C, HF, WF = 16, 256, 80, 80
D, K, HZ, WZ = 64, 3, 40, 40
NCORES = 8
BPC = B // NCORES          # batches per core
NPIX = HF * WF             # 6400
NLOW = HZ * WZ             # 1600

BF16 = ml_dtypes.bfloat16

_cache = {}


def _n_tiles():
    tiles = []
    off = 0
    while off < NPIX:
        nt = min(512, NPIX - off)
        tiles.append((off, nt))
        off += nt
    return tiles


def _build_program():
    from contextlib import ExitStack

    import concourse.bacc as bacc
    import concourse.mybir as mybir
    import concourse.tile as tile

    bf16 = mybir.dt.bfloat16
    f32 = mybir.dt.float32
    Alu = mybir.AluOpType

    nc = bacc.Bacc("TRN2", target_bir_lowering=False, debug=False)

    feat_h = nc.dram_tensor("feat", [BPC, C, NPIX], bf16, kind="ExternalInput")
    zp_h = nc.dram_tensor("zp", [BPC, D + K + 1, NLOW], bf16, kind="ExternalInput")
    u0w_h = nc.dram_tensor("u0w", [128, 512], bf16, kind="ExternalInput")
    u1w_h = nc.dram_tensor("u1w", [68, 512], bf16, kind="ExternalInput")
    out_h = nc.dram_tensor("out", [BPC, C, NPIX], bf16, kind="ExternalOutput")

    with ExitStack() as ctx:
        tc = ctx.enter_context(tile.TileContext(nc))
        wpool = ctx.enter_context(tc.tile_pool(name="w", bufs=1))
        zpl_pool = ctx.enter_context(tc.tile_pool(name="zpl", bufs=2))
        q_pool = ctx.enter_context(tc.tile_pool(name="q", bufs=1))
        zw_pool = ctx.enter_context(tc.tile_pool(name="zw", bufs=1))
        q2_pool = ctx.enter_context(tc.tile_pool(name="q2", bufs=1))
        zpu_pool = ctx.enter_context(tc.tile_pool(name="zpu", bufs=2))
        r01_pool = ctx.enter_context(tc.tile_pool(name="r01", bufs=2))
        zr_pool = ctx.enter_context(tc.tile_pool(name="zr", bufs=2))
        u0_pool = ctx.enter_context(tc.tile_pool(name="u0", bufs=2))
        u1_pool = ctx.enter_context(tc.tile_pool(name="u1", bufs=2))
        psum_pool = ctx.enter_context(tc.tile_pool(name="ps", bufs=8, space="PSUM"))
        feat_pool = ctx.enter_context(tc.tile_pool(name="f", bufs=4))
        gb_pool = ctx.enter_context(tc.tile_pool(name="gb", bufs=4))
        o_pool = ctx.enter_context(tc.tile_pool(name="o", bufs=4))

        U0 = wpool.tile([128, 512], bf16)
        U1 = wpool.tile([68, 512], bf16)
        nc.sync.dma_start(U0[:], u0w_h.ap()[:, :])
        nc.sync.dma_start(U1[:], u1w_h.ap()[:, :])

        for b in range(BPC):
            # --- stage A: load + 2x bilinear upsample of [Z; P; ones] ---
            ZPl = zpl_pool.tile([68, NLOW], bf16)
            nc.sync.dma_start(ZPl[:], zp_h.ap()[b])
            zl = ZPl[:].rearrange("p (h w) -> p h w", w=WZ)

            # W pass: out[2m] = .25 z[m-1] + .75 z[m]; out[2m+1] = .75 z[m] + .25 z[m+1]
            Q = q_pool.tile([68, NLOW], bf16)
            nc.vector.tensor_scalar_mul(Q[:], ZPl[:], 0.75)
            q3 = Q[:].rearrange("p (h w) -> p h w", w=WZ)
            Zw = zw_pool.tile([68, HZ * WF], bf16)
            zw3 = Zw[:].rearrange("p (h w) -> p h w", w=WF)
            nc.vector.scalar_tensor_tensor(
                zw3[:, :, 2::2], zl[:, :, 0:39], 0.25, q3[:, :, 1:40], Alu.mult, Alu.add
            )
            nc.vector.scalar_tensor_tensor(
                zw3[:, :, 1:79:2], zl[:, :, 1:40], 0.25, q3[:, :, 0:39], Alu.mult, Alu.add
            )
            nc.vector.scalar_tensor_tensor(
                zw3[:, :, 0:1], zl[:, :, 0:1], 0.25, q3[:, :, 0:1], Alu.mult, Alu.add
            )
            nc.vector.scalar_tensor_tensor(
                zw3[:, :, 79:80], zl[:, :, 39:40], 0.25, q3[:, :, 39:40], Alu.mult, Alu.add
            )

            # H pass
            Q2 = q2_pool.tile([68, HZ * WF], bf16)
            nc.vector.tensor_scalar_mul(Q2[:], Zw[:], 0.75)
            q23 = Q2[:].rearrange("p (h w) -> p h w", w=WF)
            ZPu = zpu_pool.tile([68, NPIX], bf16)
            zu3 = ZPu[:].rearrange("p (h w) -> p h w", w=WF)
            nc.vector.scalar_tensor_tensor(
                zu3[:, 2::2, :], zw3[:, 0:39, :], 0.25, q23[:, 1:40, :], Alu.mult, Alu.add
            )
            nc.vector.scalar_tensor_tensor(
                zu3[:, 1:79:2, :], zw3[:, 1:40, :], 0.25, q23[:, 0:39, :], Alu.mult, Alu.add
            )
            nc.vector.scalar_tensor_tensor(
                zu3[:, 0:1, :], zw3[:, 0:1, :], 0.25, q23[:, 0:1, :], Alu.mult, Alu.add
            )
            nc.vector.scalar_tensor_tensor(
                zu3[:, 79:80, :], zw3[:, 39:40, :], 0.25, q23[:, 39:40, :], Alu.mult, Alu.add
            )

            # --- router replication + u build ---
            def row_rep(ap, np_, n=64):
                # [np_, NPIX] -> [np_, n, NPIX] with a 0-step repeat free dim
                return ap.unsqueeze(1).broadcast_to((np_, n, NPIX))

            R01 = r01_pool.tile([128, NPIX], bf16)
            nc.gpsimd.dma_start(R01[:], row_rep(ZPu[64:66, :], 2))
            # ZR: rows 0..63 = broadcast P_up[2]; rows 64..127 = Z_up dup
            ZR = zr_pool.tile([128, NPIX], bf16)
            nc.gpsimd.dma_start(ZR[0:64, :], row_rep(ZPu[66:67, :], 1))
            nc.gpsimd.dma_start(ZR[64:128, :], ZPu[0:64, :])

            u0 = u0_pool.tile([128, NPIX], bf16)
            nc.vector.tensor_tensor(u0[0:64, :], ZPu[0:64, :], R01[0:64, :], Alu.mult)
            nc.vector.tensor_tensor(u0[64:128, :], ZR[64:128, :], R01[64:128, :], Alu.mult)
            u1 = u1_pool.tile([68, NPIX], bf16)
            nc.vector.tensor_tensor(u1[0:64, :], ZPu[0:64, :], ZR[0:64, :], Alu.mult)
            nc.gpsimd.tensor_copy(u1[64:68, :], ZPu[64:68, :])

            # --- stage B: matmuls + FiLM per N tile ---
            featb = feat_h.ap()[b]
            outb = out_h.ap()[b]
            for js, nt in _n_tiles():
                psums = []
                for cs in (0, 128, 256, 384):
                    ps = psum_pool.tile([128, nt], f32)
                    nc.tensor.matmul(
                        ps[:], U0[:, cs : cs + 128], u0[:, js : js + nt],
                        start=True, stop=False,
                    )
                    nc.tensor.matmul(
                        ps[:], U1[:, cs : cs + 128], u1[:, js : js + nt],
                        start=False, stop=True,
                    )
                    psums.append(ps)
                for half in range(2):
                    f = feat_pool.tile([128, nt], bf16)
                    nc.sync.dma_start(
                        f[:], featb[half * 128 : (half + 1) * 128, js : js + nt]
                    )
                    g = gb_pool.tile([128, nt], bf16)
                    nc.scalar.copy(g[:], psums[half][:])
                    bt = gb_pool.tile([128, nt], bf16)
                    nc.scalar.copy(bt[:], psums[2 + half][:])
                    o = o_pool.tile([128, nt], bf16)
                    nc.vector.tensor_tensor(o[:], f[:], g[:], Alu.mult)
                    nc.vector.tensor_tensor(o[:], o[:], bt[:], Alu.add)
                    nc.sync.dma_start(
                        outb[half * 128 : (half + 1) * 128, js : js + nt], o[:]
                    )

    nc.compile()
    return nc


def _get_program():
    if "nc" not in _cache:
        _cache["nc"] = _build_program()
    return _cache["nc"]


def _prep_weights(Wg, bg, Wb, bb):
    U = np.zeros((196, 512), np.float32)
    for k in range(3):
        U[64 * k : 64 * (k + 1), 0:256] = Wg[k].T
        U[64 * k : 64 * (k + 1), 256:512] = Wb[k].T
    U[192:195, 0:256] = bg
    U[192:195, 256:512] = bb
    U[195, 0:256] = 1.0
    U[195, 256:512] = 0.0
    Ub = U.astype(BF16)
    return np.ascontiguousarray(Ub[0:128]), np.ascontiguousarray(Ub[128:196])


def kernel(**inputs):
    import concourse.bass_utils as bass_utils

    feat = np.asarray(inputs["feat"], dtype=np.float32)
    Z = np.asarray(inputs["Z"], dtype=np.float32)
    P = np.asarray(inputs["P"], dtype=np.float32)
    U0np, U1np = _prep_weights(
        np.asarray(inputs["Wg"], dtype=np.float32),
        np.asarray(inputs["bg"], dtype=np.float32),
        np.asarray(inputs["Wb"], dtype=np.float32),
        np.asarray(inputs["bb"], dtype=np.float32),
    )

    featb = feat.reshape(B, C, NPIX).astype(BF16)
    zp = np.empty((B, D + K + 1, NLOW), np.float32)
    zp[:, :D] = Z.reshape(B, D, NLOW)
    zp[:, D : D + K] = P.reshape(B, K, NLOW)
    zp[:, D + K] = 1.0
    zpb = zp.astype(BF16)

    nc = _get_program()
    in_maps = []
    for c in range(NCORES):
        sl = slice(c * BPC, (c + 1) * BPC)
        in_maps.append(
            {
                "feat": np.ascontiguousarray(featb[sl]),
                "zp": np.ascontiguousarray(zpb[sl]),
                "u0w": U0np,
                "u1w": U1np,
            }
        )

    res = bass_utils.run_bass_kernel_spmd(nc, in_maps, core_ids=list(range(NCORES)))
    out = np.concatenate([r["out"] for r in res.results], axis=0)
    return out.astype(np.float32).reshape(B, C, HF, WF)


if __name__ == "__main__":
    import reference

    inputs = {k: np.asarray(v) for k, v in reference.setup_inputs().items()}
    out = kernel(**inputs)
    print("out", out.shape, out.dtype)


# revision 60
# speedup vs baseline: 11.0751x; 11.0751x over previous
"""Trainium2 Bass kernel for RSVFiLM (moe_routing).

Math (per batch b):
  Z_up = bilinear2x(Z[b])  [64, 80, 80]
  P_up = bilinear2x(P[b])  [3, 80, 80]
  u[j, n] rows: j in [0..195]:
     rows 0..63   : Z_up[d] * P_up[0]
     rows 64..127 : Z_up[d] * P_up[1]
     rows 128..191: Z_up[d] * P_up[2]
     rows 192..194: P_up[k]
     row 195      : 1.0
  [gamma_total; delta_beta] = U.T @ u   (U [196, 512] combines Wg/bg/+1 and Wb/bb)
  out = feat * gamma_total + delta_beta

Sharding: pure data-parallel, 2 batches per core across 8 cores. Expert
weights replicated. All device math in bf16 (fp32 PSUM accumulation).
"""

import numpy as np
import ml_dtypes

B, C, HF, WF = 16, 256, 80, 80
D, K, HZ, WZ = 64, 3, 40, 40
NCORES = 8
BPC = B // NCORES          # batches per core
NPIX = HF * WF             # 6400
NLOW = HZ * WZ             # 1600

BF16 = ml_dtypes.bfloat16

_cache = {}


GN = 800          # group width: one P-octet (10 hi rows)
FN = 400          # film/psum sub-chunk width


def _groups():
    return [(i * GN, GN) for i in range(NPIX // GN)]


def _build_program():
    from contextlib import ExitStack

    import concourse.bacc as bacc
    import concourse.mybir as mybir
    import concourse.tile as tile

    bf16 = mybir.dt.bfloat16
    f32 = mybir.dt.float32
    Alu = mybir.AluOpType

    nc = bacc.Bacc("TRN2", target_bir_lowering=False, debug=False)

    feat_h = nc.dram_tensor("feat", [BPC, C, NPIX], bf16, kind="ExternalInput")
    zp_h = nc.dram_tensor("zp", [BPC, D, NLOW], bf16, kind="ExternalInput")
    # P (+ones) packed for the 64-wide pipe: partition (b, row, octet), each
    # holding 7 halo-clamped low rows of 40
    pp8_h = nc.dram_tensor("pp8", [64, 7 * WZ], bf16, kind="ExternalInput")
    u0w_h = nc.dram_tensor("u0w", [128, 512], bf16, kind="ExternalInput")
    u1w_h = nc.dram_tensor("u1w", [68, 512], bf16, kind="ExternalInput")
    out_h = nc.dram_tensor("out", [BPC, C, NPIX], bf16, kind="ExternalOutput")

    def upsample_chunk(eng, lo, q_t, w_t, q2_t, hi, Alu, ci):
        """2x bilinear [P, 40x40] -> [P, 80x80], W pass then H pass.

        ci=0 emits W rows 0..20 / H out rows 0..39; ci=1 the rest.
        """
        zl = lo.rearrange("p (h w) -> p h w", w=WZ)
        q3 = q_t.rearrange("p (h w) -> p h w", w=WZ)
        zw3 = w_t.rearrange("p (h w) -> p h w", w=WF)
        q23 = q2_t.rearrange("p (h w) -> p h w", w=WF)
        zu3 = hi.rearrange("p (h w) -> p h w", w=WF)

        def wpass(r0, r1):
            eng.tensor_scalar_mul(q3[:, r0:r1, :], zl[:, r0:r1, :], 0.75)
            eng.scalar_tensor_tensor(
                zw3[:, r0:r1, 2::2], zl[:, r0:r1, 0:39], 0.25,
                q3[:, r0:r1, 1:40], Alu.mult, Alu.add,
            )
            eng.scalar_tensor_tensor(
                zw3[:, r0:r1, 1:79:2], zl[:, r0:r1, 1:40], 0.25,
                q3[:, r0:r1, 0:39], Alu.mult, Alu.add,
            )
            eng.scalar_tensor_tensor(
                zw3[:, r0:r1, 0:1], zl[:, r0:r1, 0:1], 0.25,
                q3[:, r0:r1, 0:1], Alu.mult, Alu.add,
            )
            eng.scalar_tensor_tensor(
                zw3[:, r0:r1, 79:80], zl[:, r0:r1, 39:40], 0.25,
                q3[:, r0:r1, 39:40], Alu.mult, Alu.add,
            )

        if ci == 0:
            # chunk 1: W rows 0..20, q2 rows 0..19, H out rows 0..39
            # (H emitted in two halves so rows 0..19 unlock early)
            wpass(0, 21)
            eng.tensor_scalar_mul(q23[:, 0:20, :], zw3[:, 0:20, :], 0.75)
            eng.scalar_tensor_tensor(
                zu3[:, 2:19:2, :], zw3[:, 0:9, :], 0.25, q23[:, 1:10, :],
                Alu.mult, Alu.add,
            )
            eng.scalar_tensor_tensor(
                zu3[:, 1:20:2, :], zw3[:, 1:11, :], 0.25, q23[:, 0:10, :],
                Alu.mult, Alu.add,
            )
            eng.scalar_tensor_tensor(
                zu3[:, 0:1, :], zw3[:, 0:1, :], 0.25, q23[:, 0:1, :],
                Alu.mult, Alu.add,
            )
            eng.scalar_tensor_tensor(
                zu3[:, 20:39:2, :], zw3[:, 9:19, :], 0.25, q23[:, 10:20, :],
                Alu.mult, Alu.add,
            )
            eng.scalar_tensor_tensor(
                zu3[:, 21:40:2, :], zw3[:, 11:21, :], 0.25, q23[:, 10:20, :],
                Alu.mult, Alu.add,
            )
        else:
            # chunk 2: W rows 21..39, q2 rows 20..39, H out rows 40..79
            wpass(21, 40)
            eng.tensor_scalar_mul(q23[:, 20:40, :], zw3[:, 20:40, :], 0.75)
            eng.scalar_tensor_tensor(
                zu3[:, 40:79:2, :], zw3[:, 19:39, :], 0.25, q23[:, 20:40, :],
                Alu.mult, Alu.add,
            )
            eng.scalar_tensor_tensor(
                zu3[:, 41:78:2, :], zw3[:, 21:40, :], 0.25, q23[:, 20:39, :],
                Alu.mult, Alu.add,
            )
            eng.scalar_tensor_tensor(
                zu3[:, 79:80, :], zw3[:, 39:40, :], 0.25, q23[:, 39:40, :],
                Alu.mult, Alu.add,
            )

    def upsample_chunk_tt(eng, lo, q_t, r_t, w_t, q2_t, r2_t, hi, Alu, ci):
        """Like upsample_chunk but STT-free (TS+TT only) for the Pool engine."""
        zl = lo.rearrange("p (h w) -> p h w", w=WZ)
        q3 = q_t.rearrange("p (h w) -> p h w", w=WZ)
        r3 = r_t.rearrange("p (h w) -> p h w", w=WZ)
        zw3 = w_t.rearrange("p (h w) -> p h w", w=WF)
        q23 = q2_t.rearrange("p (h w) -> p h w", w=WF)
        r23 = r2_t.rearrange("p (h w) -> p h w", w=WF)
        zu3 = hi.rearrange("p (h w) -> p h w", w=WF)

        def wpass(r0, r1):
            eng.tensor_scalar_mul(q3[:, r0:r1, :], zl[:, r0:r1, :], 0.75)
            eng.tensor_scalar_mul(r3[:, r0:r1, :], zl[:, r0:r1, :], 0.25)
            eng.tensor_tensor(
                zw3[:, r0:r1, 2::2], r3[:, r0:r1, 0:39], q3[:, r0:r1, 1:40], Alu.add
            )
            eng.tensor_tensor(
                zw3[:, r0:r1, 1:79:2], r3[:, r0:r1, 1:40], q3[:, r0:r1, 0:39], Alu.add
            )
            eng.tensor_tensor(
                zw3[:, r0:r1, 0:1], r3[:, r0:r1, 0:1], q3[:, r0:r1, 0:1], Alu.add
            )
            eng.tensor_tensor(
                zw3[:, r0:r1, 79:80], r3[:, r0:r1, 39:40], q3[:, r0:r1, 39:40], Alu.add
            )

        if ci == 0:
            wpass(0, 21)
            eng.tensor_scalar_mul(q23[:, 0:20, :], zw3[:, 0:20, :], 0.75)
            eng.tensor_scalar_mul(r23[:, 0:21, :], zw3[:, 0:21, :], 0.25)
            eng.tensor_tensor(
                zu3[:, 2:39:2, :], r23[:, 0:19, :], q23[:, 1:20, :], Alu.add
            )
            eng.tensor_tensor(
                zu3[:, 1:40:2, :], r23[:, 1:21, :], q23[:, 0:20, :], Alu.add
            )
            eng.tensor_tensor(zu3[:, 0:1, :], r23[:, 0:1, :], q23[:, 0:1, :], Alu.add)
        else:
            wpass(21, 40)
            eng.tensor_scalar_mul(q23[:, 20:40, :], zw3[:, 20:40, :], 0.75)
            eng.tensor_scalar_mul(r23[:, 21:40, :], zw3[:, 21:40, :], 0.25)
            eng.tensor_tensor(
                zu3[:, 40:79:2, :], r23[:, 19:39, :], q23[:, 20:40, :], Alu.add
            )
            eng.tensor_tensor(
                zu3[:, 41:78:2, :], r23[:, 21:40, :], q23[:, 20:39, :], Alu.add
            )
            eng.tensor_tensor(
                zu3[:, 79:80, :], r23[:, 39:40, :], q23[:, 39:40, :], Alu.add
            )

    def row_rep(ap, np_, fd, n=64):
        # [np_, fd] -> [np_, n, fd] with a 0-step repeat free dim
        return ap.unsqueeze(1).broadcast_to((np_, n, fd))

    with ExitStack() as ctx:
        tc = ctx.enter_context(tile.TileContext(nc))
        wpool = ctx.enter_context(tc.tile_pool(name="w", bufs=1))
        ppool = ctx.enter_context(tc.tile_pool(name="pp", bufs=1))
        zzl_pool = ctx.enter_context(tc.tile_pool(name="zzl", bufs=2))
        q_pool = ctx.enter_context(tc.tile_pool(name="q", bufs=1))
        zw_pool = ctx.enter_context(tc.tile_pool(name="zw", bufs=1))
        q2_pool = ctx.enter_context(tc.tile_pool(name="q2", bufs=1))
        zzu_pool = ctx.enter_context(tc.tile_pool(name="zzu", bufs=2))
        r01_pool = ctx.enter_context(tc.tile_pool(name="r01", bufs=3))
        u0_pool = ctx.enter_context(tc.tile_pool(name="u0", bufs=3))
        u1_pool = ctx.enter_context(tc.tile_pool(name="u1", bufs=3))
        psum_pool = ctx.enter_context(tc.tile_pool(name="ps", bufs=4, space="PSUM"))
        feat_pool = ctx.enter_context(tc.tile_pool(name="f", bufs=4))
        gb_pool = ctx.enter_context(tc.tile_pool(name="gb", bufs=4))
        o_pool = ctx.enter_context(tc.tile_pool(name="o", bufs=4))

        U0 = wpool.tile([128, 512], bf16)
        U1 = wpool.tile([68, 512], bf16)
        nc.sync.dma_start(U0[:], u0w_h.ap()[:, :])
        nc.sync.dma_start(U1[:], u1w_h.ap()[:, :])

        # --- P pipe (once per core): partition-packed 64-wide, h on partitions.
        # Partition (b, row, octet o) holds 7 halo-clamped low rows; output is
        # that octet's 10 hi rows. Host pre-clamps, so no edge ops in H.
        Ppk = ppool.tile([64, 7 * WZ], bf16)
        nc.sync.dma_start(Ppk[:], pp8_h.ap()[:, :])
        Pqk = ppool.tile([64, 7 * WZ], bf16)
        Pwk = ppool.tile([64, 7 * WF], bf16)
        Pq2k = ppool.tile([64, 7 * WF], bf16)
        Puk = ppool.tile([64, 10 * WF], bf16)
        lo3 = Ppk[:].rearrange("p (h w) -> p h w", w=WZ)
        q3 = Pqk[:].rearrange("p (h w) -> p h w", w=WZ)
        w3 = Pwk[:].rearrange("p (h w) -> p h w", w=WF)
        q23 = Pq2k[:].rearrange("p (h w) -> p h w", w=WF)
        hi3 = Puk[:].rearrange("p (h w) -> p h w", w=WF)
        nc.vector.tensor_scalar_mul(Pqk[:], Ppk[:], 0.75)
        nc.vector.scalar_tensor_tensor(
            w3[:, :, 2::2], lo3[:, :, 0:39], 0.25, q3[:, :, 1:40], Alu.mult, Alu.add
        )
        nc.vector.scalar_tensor_tensor(
            w3[:, :, 1:79:2], lo3[:, :, 1:40], 0.25, q3[:, :, 0:39], Alu.mult, Alu.add
        )
        nc.vector.scalar_tensor_tensor(
            w3[:, :, 0:1], lo3[:, :, 0:1], 0.25, q3[:, :, 0:1], Alu.mult, Alu.add
        )
        nc.vector.scalar_tensor_tensor(
            w3[:, :, 79:80], lo3[:, :, 39:40], 0.25, q3[:, :, 39:40], Alu.mult, Alu.add
        )
        nc.vector.tensor_scalar_mul(Pq2k[:], Pwk[:], 0.75)
        nc.vector.scalar_tensor_tensor(
            hi3[:, 0:10:2, :], w3[:, 0:5, :], 0.25, q23[:, 1:6, :], Alu.mult, Alu.add
        )
        nc.vector.scalar_tensor_tensor(
            hi3[:, 1:10:2, :], w3[:, 2:7, :], 0.25, q23[:, 1:6, :], Alu.mult, Alu.add
        )
        # No unpack: groups are octet-aligned (GN=800), so broadcasts read
        # Puk directly via stride-8 partition slices.

        # --- Z pipes: z duplicated into both partition halves, 128-wide ---
        zzu_tiles = {}

        def emit_zz_dma(b):
            ZZl = zzl_pool.tile([128, NLOW], bf16, name=f"ZZl{b}")
            nc.sync.dma_start(ZZl[0:64, :], zp_h.ap()[b, 0:D])
            nc.sync.dma_start(ZZl[64:128, :], zp_h.ap()[b, 0:D])
            Zq = q_pool.tile([128, NLOW], bf16, name=f"Zq{b}", tag="Zq")
            Zw = zw_pool.tile([128, HZ * WF], bf16, name=f"Zw{b}", tag="Zw")
            Zq2 = q2_pool.tile([128, HZ * WF], bf16, name=f"Zq2{b}", tag="Zq2")
            ZZu = zzu_pool.tile([128, NPIX], bf16, name=f"ZZu{b}")
            zzu_tiles[b] = (ZZl, Zq, Zw, Zq2, ZZu)

        def emit_zz_chunk(b, ci):
            ZZl, Zq, Zw, Zq2, ZZu = zzu_tiles[b]
            upsample_chunk(nc.vector, ZZl[:], Zq[:], Zw[:], Zq2[:], ZZu[:], Alu, ci)

        u_tiles = {}
        pending_outs = []

        def emit_ubuild(b, gi):
            ZZu = zzu_tiles[b][4]
            gs, gn = _groups()[gi]
            # packed-P partitions for this group's octet: (b, octet gi, row r)
            pbase = b * 32 + gi * 4
            # router replication + u build for this group's columns
            R01 = r01_pool.tile([128, gn], bf16)
            nc.sync.dma_start(
                R01[:], row_rep(Puk[pbase : pbase + 2, :], 2, gn)
            )
            u0 = u0_pool.tile([128, gn], bf16)
            u1 = u1_pool.tile([68, gn], bf16)
            # stage broadcast P_up[2] into u0's lower half, consume it for
            # u1, then overwrite u0 (same-engine WAR)
            nc.sync.dma_start(
                u0[0:64, :], row_rep(Puk[pbase + 2 : pbase + 3, :], 1, gn)
            )
            nc.vector.tensor_tensor(
                u1[0:64, :], ZZu[0:64, gs : gs + gn], u0[0:64, :], Alu.mult
            )
            nc.sync.dma_start(u1[64:68, :], Puk[pbase : pbase + 4, :])
            nc.vector.tensor_tensor(u0[:], ZZu[:, gs : gs + gn], R01[:], Alu.mult)
            u_tiles[(b, gi)] = (u0, u1)

        def emit_group(b, gi):
            featb = feat_h.ap()[b]
            outb = out_h.ap()[b]
            gs, gn = _groups()[gi]
            u0, u1 = u_tiles.pop((b, gi))
            if True:
                # one feat load / out store per group covering both channel
                # halves: [h0(gn) | h1(gn)] on partitions 0..127
                fdram = featb[:, gs : gs + gn].rearrange("(t c) x -> c t x", t=2)
                odram = outb[:, gs : gs + gn].rearrange("(t c) x -> c t x", t=2)
                f2 = feat_pool.tile([128, 2 * gn], bf16)
                nc.sync.dma_start(f2[:].rearrange("p (t x) -> p t x", t=2), fdram)
                o2 = o_pool.tile([128, 2 * gn], bf16)
                for sub in range(0, gn, FN):
                    sn = min(FN, gn - sub)
                    for half in range(2):
                        # [gamma(sn) | pad | beta(sn)]: beta at bank boundary
                        ps = psum_pool.tile([128, 1024], f32)
                        for ci, wo in ((half, 0), (2 + half, 512)):
                            nc.tensor.matmul(
                                ps[:, wo : wo + sn],
                                U0[:, ci * 128 : ci * 128 + 128],
                                u0[:, sub : sub + sn],
                                start=True, stop=False,
                            )
                            nc.tensor.matmul(
                                ps[:, wo : wo + sn],
                                U1[:, ci * 128 : ci * 128 + 128],
                                u1[:, sub : sub + sn],
                                start=False, stop=True,
                            )
                        gbc = gb_pool.tile([128, 2 * sn], bf16)
                        src = ps[:].rearrange("p (t x) -> p t x", x=512)  # [128,2,512]
                        nc.scalar.copy(
                            gbc[:].rearrange("p (t x) -> p t x", x=sn),
                            src[:, :, 0:sn],
                        )
                        oslc = o2[:, half * gn + sub : half * gn + sub + sn]
                        fslc = f2[:, half * gn + sub : half * gn + sub + sn]
                        eng = nc.vector if half == 0 else nc.gpsimd
                        eng.tensor_tensor(oslc, fslc, gbc[:, 0:sn], Alu.mult)
                        nc.gpsimd.tensor_tensor(
                            oslc, oslc, gbc[:, sn : 2 * sn], Alu.add
                        )
                pending_outs.append(
                    (odram, o2[:].rearrange("p (t x) -> p t x", t=2))
                )

        def flush_out():
            # emit the oldest deferred out store; by now its Pool add is done,
            # so it doesn't head-of-line-block the SP queue
            if pending_outs:
                odram, osrc = pending_outs.pop(0)
                nc.sync.dma_start(odram, osrc)

        # --- interleaved, software-pipelined schedule: u-build (A) runs one
        # group ahead of matmul+FiLM (B) so the DVE queue never blocks on the
        # psum->FiLM chain; ZZ pipe chunks slot between groups.
        emit_zz_dma(0)
        emit_zz_chunk(0, 0)
        emit_ubuild(0, 0)
        emit_ubuild(0, 1)
        flush_out()
        emit_group(0, 0)
        emit_ubuild(0, 2)
        flush_out()
        emit_group(0, 1)
        emit_zz_chunk(0, 1)
        emit_ubuild(0, 3)
        flush_out()
        emit_group(0, 2)
        emit_ubuild(0, 4)
        flush_out()
        emit_group(0, 3)
        emit_zz_dma(1)
        emit_zz_chunk(1, 0)
        emit_ubuild(0, 5)
        flush_out()
        emit_group(0, 4)
        emit_ubuild(0, 6)
        flush_out()
        emit_group(0, 5)
        emit_ubuild(0, 7)
        flush_out()
        emit_group(0, 6)
        emit_ubuild(1, 0)
        flush_out()
        emit_group(0, 7)
        emit_ubuild(1, 1)
        flush_out()
        emit_group(1, 0)
        emit_zz_chunk(1, 1)
        emit_ubuild(1, 2)
        flush_out()
        emit_group(1, 1)
        emit_ubuild(1, 3)
        flush_out()
        emit_group(1, 2)
        emit_ubuild(1, 4)
        flush_out()
        emit_group(1, 3)
        emit_ubuild(1, 5)
        flush_out()
        emit_group(1, 4)
        emit_ubuild(1, 6)
        flush_out()
        emit_group(1, 5)
        emit_ubuild(1, 7)
        flush_out()
        emit_group(1, 6)
        flush_out()
        emit_group(1, 7)

        flush_out()
        flush_out()
        flush_out()
    nc.compile()
    return nc


def _get_program():
    if "nc" not in _cache:
        _cache["nc"] = _build_program()
    return _cache["nc"]


def _pack_p(P):
    """[B, K, HZ, WZ] -> per-core [64, 7*WZ]: partition (b, row, octet) holds
    7 halo-clamped low rows (rows: p0,p1,p2,ones)."""
    plow = np.empty((B, 4, HZ, WZ), np.float32)
    plow[:, :K] = P.reshape(B, K, HZ, WZ)
    plow[:, K] = 1.0
    pp = np.empty((B, 4, 8, 7, WZ), np.float32)
    for o in range(8):
        idx = np.clip(np.arange(5 * o - 1, 5 * o + 6), 0, HZ - 1)
        pp[:, :, o] = plow[:, :, idx, :]
    pp = pp.transpose(0, 2, 1, 3, 4)  # (b, octet, row, 7, WZ)
    pp = np.ascontiguousarray(pp).astype(BF16).reshape(NCORES, BPC * 4 * 8, 7 * WZ)
    return [np.ascontiguousarray(pp[c]) for c in range(NCORES)]


def _prep_weights(Wg, bg, Wb, bb):
    U = np.zeros((196, 512), np.float32)
    for k in range(3):
        U[64 * k : 64 * (k + 1), 0:256] = Wg[k].T
        U[64 * k : 64 * (k + 1), 256:512] = Wb[k].T
    U[192:195, 0:256] = bg
    U[192:195, 256:512] = bb
    U[195, 0:256] = 1.0
    U[195, 256:512] = 0.0
    Ub = U.astype(BF16)
    return np.ascontiguousarray(Ub[0:128]), np.ascontiguousarray(Ub[128:196])


def kernel(**inputs):
    import concourse.bass_utils as bass_utils

    feat = np.asarray(inputs["feat"], dtype=np.float32)
    Z = np.asarray(inputs["Z"], dtype=np.float32)
    P = np.asarray(inputs["P"], dtype=np.float32)
    U0np, U1np = _prep_weights(
        np.asarray(inputs["Wg"], dtype=np.float32),
        np.asarray(inputs["bg"], dtype=np.float32),
        np.asarray(inputs["Wb"], dtype=np.float32),
        np.asarray(inputs["bb"], dtype=np.float32),
    )

    featb = feat.reshape(B, C, NPIX).astype(BF16)
    zpb = Z.reshape(B, D, NLOW).astype(BF16)
    pp8b = _pack_p(P)

    nc = _get_program()
    in_maps = []
    for c in range(NCORES):
        sl = slice(c * BPC, (c + 1) * BPC)
        in_maps.append(
            {
                "feat": np.ascontiguousarray(featb[sl]),
                "zp": np.ascontiguousarray(zpb[sl]),
                "pp8": pp8b[c],
                "u0w": U0np,
                "u1w": U1np,
            }
        )

    res = bass_utils.run_bass_kernel_spmd(nc, in_maps, core_ids=list(range(NCORES)))
    out = np.concatenate([r["out"] for r in res.results], axis=0)
    return out.astype(np.float32).reshape(B, C, HF, WF)


if __name__ == "__main__":
    import reference

    inputs = {k: np.asarray(v) for k, v in reference.setup_inputs().items()}
    out = kernel(**inputs)
    print("out", out.shape, out.dtype)


# revision 66
# speedup vs baseline: 11.1128x; 1.0034x over previous
"""Trainium2 Bass kernel for RSVFiLM (moe_routing).

Math (per batch b):
  Z_up = bilinear2x(Z[b])  [64, 80, 80]
  P_up = bilinear2x(P[b])  [3, 80, 80]
  u[j, n] rows: j in [0..195]:
     rows 0..63   : Z_up[d] * P_up[0]
     rows 64..127 : Z_up[d] * P_up[1]
     rows 128..191: Z_up[d] * P_up[2]
     rows 192..194: P_up[k]
     row 195      : 1.0
  [gamma_total; delta_beta] = U.T @ u   (U [196, 512] combines Wg/bg/+1 and Wb/bb)
  out = feat * gamma_total + delta_beta

Sharding: pure data-parallel, 2 batches per core across 8 cores. Expert
weights replicated. All device math in bf16 (fp32 PSUM accumulation).
"""

import numpy as np
import ml_dtypes

B, C, HF, WF = 16, 256, 80, 80
D, K, HZ, WZ = 64, 3, 40, 40
NCORES = 8
BPC = B // NCORES          # batches per core
NPIX = HF * WF             # 6400
NLOW = HZ * WZ             # 1600

BF16 = ml_dtypes.bfloat16

_cache = {}


GN = 800          # group width: one P-octet (10 hi rows)
FN = 400          # film/psum sub-chunk width


def _groups():
    return [(i * GN, GN) for i in range(NPIX // GN)]


def _build_program():
    from contextlib import ExitStack

    import concourse.bacc as bacc
    import concourse.mybir as mybir
    import concourse.tile as tile

    bf16 = mybir.dt.bfloat16
    f32 = mybir.dt.float32
    Alu = mybir.AluOpType

    nc = bacc.Bacc("TRN2", target_bir_lowering=False, debug=False)

    feat_h = nc.dram_tensor("feat", [BPC, C, NPIX], bf16, kind="ExternalInput")
    zp_h = nc.dram_tensor("zp", [BPC, D, NLOW], bf16, kind="ExternalInput")
    # P (+ones) packed for the 64-wide pipe: partition (b, row, octet), each
    # holding 7 halo-clamped low rows of 40
    pp8_h = nc.dram_tensor("pp8", [64, 7 * WZ], bf16, kind="ExternalInput")
    u0w_h = nc.dram_tensor("u0w", [128, 512], bf16, kind="ExternalInput")
    u1w_h = nc.dram_tensor("u1w", [68, 512], bf16, kind="ExternalInput")
    out_h = nc.dram_tensor("out", [BPC, C, NPIX], bf16, kind="ExternalOutput")

    def upsample_chunk(eng, lo, q_t, w_t, q2_t, hi, Alu, ci):
        """2x bilinear [P, 40x40] -> [P, 80x80], W pass then H pass.

        ci=0 emits W rows 0..20 / H out rows 0..39; ci=1 the rest.
        """
        zl = lo.rearrange("p (h w) -> p h w", w=WZ)
        q3 = q_t.rearrange("p (h w) -> p h w", w=WZ)
        zw3 = w_t.rearrange("p (h w) -> p h w", w=WF)
        q23 = q2_t.rearrange("p (h w) -> p h w", w=WF)
        zu3 = hi.rearrange("p (h w) -> p h w", w=WF)

        def wpass(r0, r1):
            eng.tensor_scalar_mul(q3[:, r0:r1, :], zl[:, r0:r1, :], 0.75)
            eng.scalar_tensor_tensor(
                zw3[:, r0:r1, 2::2], zl[:, r0:r1, 0:39], 0.25,
                q3[:, r0:r1, 1:40], Alu.mult, Alu.add,
            )
            eng.scalar_tensor_tensor(
                zw3[:, r0:r1, 1:79:2], zl[:, r0:r1, 1:40], 0.25,
                q3[:, r0:r1, 0:39], Alu.mult, Alu.add,
            )
            eng.scalar_tensor_tensor(
                zw3[:, r0:r1, 0:1], zl[:, r0:r1, 0:1], 0.25,
                q3[:, r0:r1, 0:1], Alu.mult, Alu.add,
            )
            eng.scalar_tensor_tensor(
                zw3[:, r0:r1, 79:80], zl[:, r0:r1, 39:40], 0.25,
                q3[:, r0:r1, 39:40], Alu.mult, Alu.add,
            )

        if ci == 0:
            # chunk 1: W rows 0..20, q2 rows 0..19, H out rows 0..39
            # (H emitted in two halves so rows 0..19 unlock early)
            wpass(0, 21)
            eng.tensor_scalar_mul(q23[:, 0:20, :], zw3[:, 0:20, :], 0.75)
            eng.scalar_tensor_tensor(
                zu3[:, 2:19:2, :], zw3[:, 0:9, :], 0.25, q23[:, 1:10, :],
                Alu.mult, Alu.add,
            )
            eng.scalar_tensor_tensor(
                zu3[:, 1:20:2, :], zw3[:, 1:11, :], 0.25, q23[:, 0:10, :],
                Alu.mult, Alu.add,
            )
            eng.scalar_tensor_tensor(
                zu3[:, 0:1, :], zw3[:, 0:1, :], 0.25, q23[:, 0:1, :],
                Alu.mult, Alu.add,
            )
            eng.scalar_tensor_tensor(
                zu3[:, 20:39:2, :], zw3[:, 9:19, :], 0.25, q23[:, 10:20, :],
                Alu.mult, Alu.add,
            )
            eng.scalar_tensor_tensor(
                zu3[:, 21:40:2, :], zw3[:, 11:21, :], 0.25, q23[:, 10:20, :],
                Alu.mult, Alu.add,
            )
        else:
            # chunk 2: W rows 21..39, q2 rows 20..39, H out rows 40..79
            wpass(21, 40)
            eng.tensor_scalar_mul(q23[:, 20:40, :], zw3[:, 20:40, :], 0.75)
            eng.scalar_tensor_tensor(
                zu3[:, 40:79:2, :], zw3[:, 19:39, :], 0.25, q23[:, 20:40, :],
                Alu.mult, Alu.add,
            )
            eng.scalar_tensor_tensor(
                zu3[:, 41:78:2, :], zw3[:, 21:40, :], 0.25, q23[:, 20:39, :],
                Alu.mult, Alu.add,
            )
            eng.scalar_tensor_tensor(
                zu3[:, 79:80, :], zw3[:, 39:40, :], 0.25, q23[:, 39:40, :],
                Alu.mult, Alu.add,
            )

    def upsample_chunk_tt(eng, lo, q_t, r_t, w_t, q2_t, r2_t, hi, Alu, ci):
        """Like upsample_chunk but STT-free (TS+TT only) for the Pool engine."""
        zl = lo.rearrange("p (h w) -> p h w", w=WZ)
        q3 = q_t.rearrange("p (h w) -> p h w", w=WZ)
        r3 = r_t.rearrange("p (h w) -> p h w", w=WZ)
        zw3 = w_t.rearrange("p (h w) -> p h w", w=WF)
        q23 = q2_t.rearrange("p (h w) -> p h w", w=WF)
        r23 = r2_t.rearrange("p (h w) -> p h w", w=WF)
        zu3 = hi.rearrange("p (h w) -> p h w", w=WF)

        def wpass(r0, r1):
            eng.tensor_scalar_mul(q3[:, r0:r1, :], zl[:, r0:r1, :], 0.75)
            eng.tensor_scalar_mul(r3[:, r0:r1, :], zl[:, r0:r1, :], 0.25)
            eng.tensor_tensor(
                zw3[:, r0:r1, 2::2], r3[:, r0:r1, 0:39], q3[:, r0:r1, 1:40], Alu.add
            )
            eng.tensor_tensor(
                zw3[:, r0:r1, 1:79:2], r3[:, r0:r1, 1:40], q3[:, r0:r1, 0:39], Alu.add
            )
            eng.tensor_tensor(
                zw3[:, r0:r1, 0:1], r3[:, r0:r1, 0:1], q3[:, r0:r1, 0:1], Alu.add
            )
            eng.tensor_tensor(
                zw3[:, r0:r1, 79:80], r3[:, r0:r1, 39:40], q3[:, r0:r1, 39:40], Alu.add
            )

        if ci == 0:
            wpass(0, 21)
            eng.tensor_scalar_mul(q23[:, 0:20, :], zw3[:, 0:20, :], 0.75)
            eng.tensor_scalar_mul(r23[:, 0:21, :], zw3[:, 0:21, :], 0.25)
            eng.tensor_tensor(
                zu3[:, 2:39:2, :], r23[:, 0:19, :], q23[:, 1:20, :], Alu.add
            )
            eng.tensor_tensor(
                zu3[:, 1:40:2, :], r23[:, 1:21, :], q23[:, 0:20, :], Alu.add
            )
            eng.tensor_tensor(zu3[:, 0:1, :], r23[:, 0:1, :], q23[:, 0:1, :], Alu.add)
        else:
            wpass(21, 40)
            eng.tensor_scalar_mul(q23[:, 20:40, :], zw3[:, 20:40, :], 0.75)
            eng.tensor_scalar_mul(r23[:, 21:40, :], zw3[:, 21:40, :], 0.25)
            eng.tensor_tensor(
                zu3[:, 40:79:2, :], r23[:, 19:39, :], q23[:, 20:40, :], Alu.add
            )
            eng.tensor_tensor(
                zu3[:, 41:78:2, :], r23[:, 21:40, :], q23[:, 20:39, :], Alu.add
            )
            eng.tensor_tensor(
                zu3[:, 79:80, :], r23[:, 39:40, :], q23[:, 39:40, :], Alu.add
            )

    def row_rep(ap, np_, fd, n=64):
        # [np_, fd] -> [np_, n, fd] with a 0-step repeat free dim
        return ap.unsqueeze(1).broadcast_to((np_, n, fd))

    with ExitStack() as ctx:
        tc = ctx.enter_context(tile.TileContext(nc))
        wpool = ctx.enter_context(tc.tile_pool(name="w", bufs=1))
        ppool = ctx.enter_context(tc.tile_pool(name="pp", bufs=1))
        zzl_pool = ctx.enter_context(tc.tile_pool(name="zzl", bufs=2))
        q_pool = ctx.enter_context(tc.tile_pool(name="q", bufs=1))
        zw_pool = ctx.enter_context(tc.tile_pool(name="zw", bufs=1))
        q2_pool = ctx.enter_context(tc.tile_pool(name="q2", bufs=1))
        zzu_pool = ctx.enter_context(tc.tile_pool(name="zzu", bufs=2))
        r01_pool = ctx.enter_context(tc.tile_pool(name="r01", bufs=3))
        u0_pool = ctx.enter_context(tc.tile_pool(name="u0", bufs=3))
        u1_pool = ctx.enter_context(tc.tile_pool(name="u1", bufs=3))
        psum_pool = ctx.enter_context(tc.tile_pool(name="ps", bufs=4, space="PSUM"))
        feat_pool = ctx.enter_context(tc.tile_pool(name="f", bufs=4))
        gb_pool = ctx.enter_context(tc.tile_pool(name="gb", bufs=4))
        o_pool = ctx.enter_context(tc.tile_pool(name="o", bufs=4))

        U0 = wpool.tile([128, 512], bf16)
        U1 = wpool.tile([68, 512], bf16)
        nc.sync.dma_start(U0[:], u0w_h.ap()[:, :])
        nc.sync.dma_start(U1[:], u1w_h.ap()[:, :])

        # --- P pipe (once per core): partition-packed 64-wide, h on partitions.
        # Partition (b, row, octet o) holds 7 halo-clamped low rows; output is
        # that octet's 10 hi rows. Host pre-clamps, so no edge ops in H.
        Ppk = ppool.tile([64, 7 * WZ], bf16)
        nc.sync.dma_start(Ppk[:], pp8_h.ap()[:, :])
        Pqk = ppool.tile([64, 7 * WZ], bf16)
        Pwk = ppool.tile([64, 7 * WF], bf16)
        Pq2k = ppool.tile([64, 7 * WF], bf16)
        Puk = ppool.tile([64, 10 * WF], bf16)
        lo3 = Ppk[:].rearrange("p (h w) -> p h w", w=WZ)
        q3 = Pqk[:].rearrange("p (h w) -> p h w", w=WZ)
        w3 = Pwk[:].rearrange("p (h w) -> p h w", w=WF)
        q23 = Pq2k[:].rearrange("p (h w) -> p h w", w=WF)
        hi3 = Puk[:].rearrange("p (h w) -> p h w", w=WF)
        nc.vector.tensor_scalar_mul(Pqk[:], Ppk[:], 0.75)
        nc.vector.scalar_tensor_tensor(
            w3[:, :, 2::2], lo3[:, :, 0:39], 0.25, q3[:, :, 1:40], Alu.mult, Alu.add
        )
        nc.vector.scalar_tensor_tensor(
            w3[:, :, 1:79:2], lo3[:, :, 1:40], 0.25, q3[:, :, 0:39], Alu.mult, Alu.add
        )
        nc.vector.scalar_tensor_tensor(
            w3[:, :, 0:1], lo3[:, :, 0:1], 0.25, q3[:, :, 0:1], Alu.mult, Alu.add
        )
        nc.vector.scalar_tensor_tensor(
            w3[:, :, 79:80], lo3[:, :, 39:40], 0.25, q3[:, :, 39:40], Alu.mult, Alu.add
        )
        nc.vector.tensor_scalar_mul(Pq2k[:], Pwk[:], 0.75)
        nc.vector.scalar_tensor_tensor(
            hi3[:, 0:10:2, :], w3[:, 0:5, :], 0.25, q23[:, 1:6, :], Alu.mult, Alu.add
        )
        nc.vector.scalar_tensor_tensor(
            hi3[:, 1:10:2, :], w3[:, 2:7, :], 0.25, q23[:, 1:6, :], Alu.mult, Alu.add
        )
        # No unpack: groups are octet-aligned (GN=800), so broadcasts read
        # Puk directly via contiguous partition slices.

        # --- Z pipes: z duplicated into both partition halves, 128-wide ---
        zzu_tiles = {}

        def emit_zz_dma(b):
            ZZl = zzl_pool.tile([128, NLOW], bf16, name=f"ZZl{b}")
            nc.sync.dma_start(ZZl[0:64, :], zp_h.ap()[b, 0:D])
            nc.sync.dma_start(ZZl[64:128, :], zp_h.ap()[b, 0:D])
            Zq = q_pool.tile([128, NLOW], bf16, name=f"Zq{b}", tag="Zq")
            Zw = zw_pool.tile([128, HZ * WF], bf16, name=f"Zw{b}", tag="Zw")
            Zq2 = q2_pool.tile([128, HZ * WF], bf16, name=f"Zq2{b}", tag="Zq2")
            ZZu = zzu_pool.tile([128, NPIX], bf16, name=f"ZZu{b}")
            zzu_tiles[b] = (ZZl, Zq, Zw, Zq2, ZZu)

        def emit_zz_chunk(b, ci):
            ZZl, Zq, Zw, Zq2, ZZu = zzu_tiles[b]
            upsample_chunk(nc.vector, ZZl[:], Zq[:], Zw[:], Zq2[:], ZZu[:], Alu, ci)

        u_tiles = {}
        pending_outs = []
        pair_tiles = {}

        def emit_ubuild(b, gi):
            ZZu = zzu_tiles[b][4]
            gs, gn = _groups()[gi]
            # packed-P partitions for this group's octet: (b, octet gi, row r)
            pbase = b * 32 + gi * 4
            # router replication + u build for this group's columns
            R01 = r01_pool.tile([128, gn], bf16)
            nc.sync.dma_start(
                R01[:], row_rep(Puk[pbase : pbase + 2, :], 2, gn)
            )
            u0 = u0_pool.tile([128, gn], bf16)
            u1 = u1_pool.tile([68, gn], bf16)
            # stage broadcast P_up[2] into u0's lower half, consume it for
            # u1, then overwrite u0 (same-engine WAR)
            nc.sync.dma_start(
                u0[0:64, :], row_rep(Puk[pbase + 2 : pbase + 3, :], 1, gn)
            )
            nc.vector.tensor_tensor(
                u1[0:64, :], ZZu[0:64, gs : gs + gn], u0[0:64, :], Alu.mult
            )
            nc.sync.dma_start(u1[64:68, :], Puk[pbase : pbase + 4, :])
            nc.vector.tensor_tensor(u0[:], ZZu[:, gs : gs + gn], R01[:], Alu.mult)
            u_tiles[(b, gi)] = (u0, u1)

        def emit_group(b, gi):
            featb = feat_h.ap()[b]
            outb = out_h.ap()[b]
            gs, gn = _groups()[gi]
            u0, u1 = u_tiles.pop((b, gi))
            if True:
                # one feat load / out store per PAIR of groups, both channel
                # halves: tile layout [h0(2gn) | h1(2gn)] on partitions 0..127
                if gi % 2 == 0:
                    pgs = gs
                    fdram = featb[:, pgs : pgs + 2 * gn].rearrange(
                        "(t c) x -> c t x", t=2
                    )
                    f2 = feat_pool.tile([128, 4 * gn], bf16)
                    nc.sync.dma_start(
                        f2[:].rearrange("p (t x) -> p t x", t=2), fdram
                    )
                    o2 = o_pool.tile([128, 4 * gn], bf16)
                    pair_tiles[b] = (f2, o2, pgs)
                f2, o2, pgs = pair_tiles[b]
                goff = gs - pgs  # 0 or gn within the pair span
                for sub in range(0, gn, FN):
                    sn = min(FN, gn - sub)
                    for half in range(2):
                        # [gamma(sn) | pad | beta(sn)]: beta at bank boundary
                        ps = psum_pool.tile([128, 1024], f32)
                        for ci, wo in ((half, 0), (2 + half, 512)):
                            nc.tensor.matmul(
                                ps[:, wo : wo + sn],
                                U0[:, ci * 128 : ci * 128 + 128],
                                u0[:, sub : sub + sn],
                                start=True, stop=False,
                            )
                            nc.tensor.matmul(
                                ps[:, wo : wo + sn],
                                U1[:, ci * 128 : ci * 128 + 128],
                                u1[:, sub : sub + sn],
                                start=False, stop=True,
                            )
                        gbc = gb_pool.tile([128, 2 * sn], bf16)
                        src = ps[:].rearrange("p (t x) -> p t x", x=512)  # [128,2,512]
                        nc.scalar.copy(
                            gbc[:].rearrange("p (t x) -> p t x", x=sn),
                            src[:, :, 0:sn],
                        )
                        co = half * 2 * gn + goff + sub
                        oslc = o2[:, co : co + sn]
                        fslc = f2[:, co : co + sn]
                        eng = nc.vector if half == 0 else nc.gpsimd
                        eng.tensor_tensor(oslc, fslc, gbc[:, 0:sn], Alu.mult)
                        nc.gpsimd.tensor_tensor(
                            oslc, oslc, gbc[:, sn : 2 * sn], Alu.add
                        )
                if gi % 2 == 1:
                    odram = outb[:, pgs : pgs + 2 * gn].rearrange(
                        "(t c) x -> c t x", t=2
                    )
                    pending_outs.append(
                        (odram, o2[:].rearrange("p (t x) -> p t x", t=2))
                    )

        def flush_out():
            # emit the oldest deferred out store; by now its Pool add is done,
            # so it doesn't head-of-line-block the SP queue
            if pending_outs:
                odram, osrc = pending_outs.pop(0)
                nc.sync.dma_start(odram, osrc)

        # --- interleaved, software-pipelined schedule: u-build (A) runs one
        # group ahead of matmul+FiLM (B) so the DVE queue never blocks on the
        # psum->FiLM chain; ZZ pipe chunks slot between groups.
        emit_zz_dma(0)
        emit_zz_chunk(0, 0)
        emit_ubuild(0, 0)
        emit_ubuild(0, 1)
        flush_out()
        emit_group(0, 0)
        emit_ubuild(0, 2)
        flush_out()
        emit_group(0, 1)
        emit_zz_chunk(0, 1)
        emit_ubuild(0, 3)
        flush_out()
        emit_group(0, 2)
        emit_ubuild(0, 4)
        flush_out()
        emit_group(0, 3)
        emit_zz_dma(1)
        emit_zz_chunk(1, 0)
        emit_ubuild(0, 5)
        flush_out()
        emit_group(0, 4)
        emit_ubuild(0, 6)
        flush_out()
        emit_group(0, 5)
        emit_ubuild(0, 7)
        flush_out()
        emit_group(0, 6)
        emit_ubuild(1, 0)
        flush_out()
        emit_group(0, 7)
        emit_ubuild(1, 1)
        flush_out()
        emit_group(1, 0)
        emit_zz_chunk(1, 1)
        emit_ubuild(1, 2)
        flush_out()
        emit_group(1, 1)
        emit_ubuild(1, 3)
        flush_out()
        emit_group(1, 2)
        emit_ubuild(1, 4)
        flush_out()
        emit_group(1, 3)
        emit_ubuild(1, 5)
        flush_out()
        emit_group(1, 4)
        emit_ubuild(1, 6)
        flush_out()
        emit_group(1, 5)
        emit_ubuild(1, 7)
        flush_out()
        emit_group(1, 6)
        flush_out()
        emit_group(1, 7)
        flush_out()
        flush_out()
    nc.compile()
    return nc


def _get_program():
    if "nc" not in _cache:
        _cache["nc"] = _build_program()
    return _cache["nc"]


def _pack_p(P):
    """[B, K, HZ, WZ] -> per-core [64, 7*WZ]: partition (b, row, octet) holds
    7 halo-clamped low rows (rows: p0,p1,p2,ones)."""
    plow = np.empty((B, 4, HZ, WZ), np.float32)
    plow[:, :K] = P.reshape(B, K, HZ, WZ)
    plow[:, K] = 1.0
    pp = np.empty((B, 4, 8, 7, WZ), np.float32)
    for o in range(8):
        idx = np.clip(np.arange(5 * o - 1, 5 * o + 6), 0, HZ - 1)
        pp[:, :, o] = plow[:, :, idx, :]
    pp = pp.transpose(0, 2, 1, 3, 4)  # (b, octet, row, 7, WZ)
    pp = np.ascontiguousarray(pp).astype(BF16).reshape(NCORES, BPC * 4 * 8, 7 * WZ)
    return [np.ascontiguousarray(pp[c]) for c in range(NCORES)]


def _prep_weights(Wg, bg, Wb, bb):
    U = np.zeros((196, 512), np.float32)
    for k in range(3):
        U[64 * k : 64 * (k + 1), 0:256] = Wg[k].T
        U[64 * k : 64 * (k + 1), 256:512] = Wb[k].T
    U[192:195, 0:256] = bg
    U[192:195, 256:512] = bb
    U[195, 0:256] = 1.0
    U[195, 256:512] = 0.0
    Ub = U.astype(BF16)
    return np.ascontiguousarray(Ub[0:128]), np.ascontiguousarray(Ub[128:196])


def kernel(**inputs):
    import concourse.bass_utils as bass_utils

    feat = np.asarray(inputs["feat"], dtype=np.float32)
    Z = np.asarray(inputs["Z"], dtype=np.float32)
    P = np.asarray(inputs["P"], dtype=np.float32)
    U0np, U1np = _prep_weights(
        np.asarray(inputs["Wg"], dtype=np.float32),
        np.asarray(inputs["bg"], dtype=np.float32),
        np.asarray(inputs["Wb"], dtype=np.float32),
        np.asarray(inputs["bb"], dtype=np.float32),
    )

    featb = feat.reshape(B, C, NPIX).astype(BF16)
    zpb = Z.reshape(B, D, NLOW).astype(BF16)
    pp8b = _pack_p(P)

    nc = _get_program()
    in_maps = []
    for c in range(NCORES):
        sl = slice(c * BPC, (c + 1) * BPC)
        in_maps.append(
            {
                "feat": np.ascontiguousarray(featb[sl]),
                "zp": np.ascontiguousarray(zpb[sl]),
                "pp8": pp8b[c],
                "u0w": U0np,
                "u1w": U1np,
            }
        )

    res = bass_utils.run_bass_kernel_spmd(nc, in_maps, core_ids=list(range(NCORES)))
    out = np.concatenate([r["out"] for r in res.results], axis=0)
    return out.astype(np.float32).reshape(B, C, HF, WF)


if __name__ == "__main__":
    import reference

    inputs = {k: np.asarray(v) for k, v in reference.setup_inputs().items()}
    out = kernel(**inputs)
    print("out", out.shape, out.dtype)


# revision 68
# speedup vs baseline: 11.7060x; 1.0534x over previous
"""Trainium2 Bass kernel for RSVFiLM (moe_routing).

Math (per batch b):
  Z_up = bilinear2x(Z[b])  [64, 80, 80]
  P_up = bilinear2x(P[b])  [3, 80, 80]
  u[j, n] rows: j in [0..195]:
     rows 0..63   : Z_up[d] * P_up[0]
     rows 64..127 : Z_up[d] * P_up[1]
     rows 128..191: Z_up[d] * P_up[2]
     rows 192..194: P_up[k]
     row 195      : 1.0
  [gamma_total; delta_beta] = U.T @ u   (U [196, 512] combines Wg/bg/+1 and Wb/bb)
  out = feat * gamma_total + delta_beta

Sharding: pure data-parallel, 2 batches per core across 8 cores. Expert
weights replicated. All device math in bf16 (fp32 PSUM accumulation).
"""

import numpy as np
import ml_dtypes

B, C, HF, WF = 16, 256, 80, 80
D, K, HZ, WZ = 64, 3, 40, 40
NCORES = 8
BPC = B // NCORES          # batches per core
NPIX = HF * WF             # 6400
NLOW = HZ * WZ             # 1600

BF16 = ml_dtypes.bfloat16

_cache = {}


GN = 800          # group width: one P-octet (10 hi rows)
FN = 400          # film/psum sub-chunk width


def _groups():
    return [(i * GN, GN) for i in range(NPIX // GN)]


def _build_program():
    from contextlib import ExitStack

    import concourse.bacc as bacc
    import concourse.mybir as mybir
    import concourse.tile as tile

    bf16 = mybir.dt.bfloat16
    f32 = mybir.dt.float32
    Alu = mybir.AluOpType

    nc = bacc.Bacc("TRN2", target_bir_lowering=False, debug=False)

    feat_h = nc.dram_tensor("feat", [BPC, C, NPIX], bf16, kind="ExternalInput")
    zp_h = nc.dram_tensor("zp", [BPC, D, NLOW], bf16, kind="ExternalInput")
    # P (+ones) packed for the 64-wide pipe: partition (b, row, octet), each
    # holding 7 halo-clamped low rows of 40
    pp8_h = nc.dram_tensor("pp8", [64, 7 * WZ], bf16, kind="ExternalInput")
    u0w_h = nc.dram_tensor("u0w", [128, 512], bf16, kind="ExternalInput")
    u1w_h = nc.dram_tensor("u1w", [68, 512], bf16, kind="ExternalInput")
    out_h = nc.dram_tensor("out", [BPC, C, NPIX], bf16, kind="ExternalOutput")

    def upsample_chunk(eng, lo, q_t, w_t, q2_t, hi, Alu, ci):
        """2x bilinear [P, 40x40] -> [P, 80x80], W pass then H pass.

        ci=0 emits W rows 0..20 / H out rows 0..39; ci=1 the rest.
        """
        zl = lo.rearrange("p (h w) -> p h w", w=WZ)
        q3 = q_t.rearrange("p (h w) -> p h w", w=WZ)
        zw3 = w_t.rearrange("p (h w) -> p h w", w=WF)
        q23 = q2_t.rearrange("p (h w) -> p h w", w=WF)
        zu3 = hi.rearrange("p (h w) -> p h w", w=WF)

        def wpass(r0, r1):
            eng.tensor_scalar_mul(q3[:, r0:r1, :], zl[:, r0:r1, :], 0.75)
            eng.scalar_tensor_tensor(
                zw3[:, r0:r1, 2::2], zl[:, r0:r1, 0:39], 0.25,
                q3[:, r0:r1, 1:40], Alu.mult, Alu.add,
            )
            eng.scalar_tensor_tensor(
                zw3[:, r0:r1, 1:79:2], zl[:, r0:r1, 1:40], 0.25,
                q3[:, r0:r1, 0:39], Alu.mult, Alu.add,
            )
            eng.scalar_tensor_tensor(
                zw3[:, r0:r1, 0:1], zl[:, r0:r1, 0:1], 0.25,
                q3[:, r0:r1, 0:1], Alu.mult, Alu.add,
            )
            eng.scalar_tensor_tensor(
                zw3[:, r0:r1, 79:80], zl[:, r0:r1, 39:40], 0.25,
                q3[:, r0:r1, 39:40], Alu.mult, Alu.add,
            )

        if ci == 0:
            # chunk 1: W rows 0..20, q2 rows 0..19, H out rows 0..39
            # (H emitted in two halves so rows 0..19 unlock early)
            wpass(0, 21)
            eng.tensor_scalar_mul(q23[:, 0:20, :], zw3[:, 0:20, :], 0.75)
            eng.scalar_tensor_tensor(
                zu3[:, 2:19:2, :], zw3[:, 0:9, :], 0.25, q23[:, 1:10, :],
                Alu.mult, Alu.add,
            )
            eng.scalar_tensor_tensor(
                zu3[:, 1:20:2, :], zw3[:, 1:11, :], 0.25, q23[:, 0:10, :],
                Alu.mult, Alu.add,
            )
            eng.scalar_tensor_tensor(
                zu3[:, 0:1, :], zw3[:, 0:1, :], 0.25, q23[:, 0:1, :],
                Alu.mult, Alu.add,
            )
            eng.scalar_tensor_tensor(
                zu3[:, 20:39:2, :], zw3[:, 9:19, :], 0.25, q23[:, 10:20, :],
                Alu.mult, Alu.add,
            )
            eng.scalar_tensor_tensor(
                zu3[:, 21:40:2, :], zw3[:, 11:21, :], 0.25, q23[:, 10:20, :],
                Alu.mult, Alu.add,
            )
        else:
            # chunk 2: W rows 21..39, q2 rows 20..39, H out rows 40..79
            wpass(21, 40)
            eng.tensor_scalar_mul(q23[:, 20:40, :], zw3[:, 20:40, :], 0.75)
            eng.scalar_tensor_tensor(
                zu3[:, 40:79:2, :], zw3[:, 19:39, :], 0.25, q23[:, 20:40, :],
                Alu.mult, Alu.add,
            )
            eng.scalar_tensor_tensor(
                zu3[:, 41:78:2, :], zw3[:, 21:40, :], 0.25, q23[:, 20:39, :],
                Alu.mult, Alu.add,
            )
            eng.scalar_tensor_tensor(
                zu3[:, 79:80, :], zw3[:, 39:40, :], 0.25, q23[:, 39:40, :],
                Alu.mult, Alu.add,
            )

    def upsample_chunk_tt(eng, lo, q_t, r_t, w_t, q2_t, r2_t, hi, Alu, ci):
        """Like upsample_chunk but STT-free (TS+TT only) for the Pool engine."""
        zl = lo.rearrange("p (h w) -> p h w", w=WZ)
        q3 = q_t.rearrange("p (h w) -> p h w", w=WZ)
        r3 = r_t.rearrange("p (h w) -> p h w", w=WZ)
        zw3 = w_t.rearrange("p (h w) -> p h w", w=WF)
        q23 = q2_t.rearrange("p (h w) -> p h w", w=WF)
        r23 = r2_t.rearrange("p (h w) -> p h w", w=WF)
        zu3 = hi.rearrange("p (h w) -> p h w", w=WF)

        def wpass(r0, r1):
            eng.tensor_scalar_mul(q3[:, r0:r1, :], zl[:, r0:r1, :], 0.75)
            eng.tensor_scalar_mul(r3[:, r0:r1, :], zl[:, r0:r1, :], 0.25)
            eng.tensor_tensor(
                zw3[:, r0:r1, 2::2], r3[:, r0:r1, 0:39], q3[:, r0:r1, 1:40], Alu.add
            )
            eng.tensor_tensor(
                zw3[:, r0:r1, 1:79:2], r3[:, r0:r1, 1:40], q3[:, r0:r1, 0:39], Alu.add
            )
            eng.tensor_tensor(
                zw3[:, r0:r1, 0:1], r3[:, r0:r1, 0:1], q3[:, r0:r1, 0:1], Alu.add
            )
            eng.tensor_tensor(
                zw3[:, r0:r1, 79:80], r3[:, r0:r1, 39:40], q3[:, r0:r1, 39:40], Alu.add
            )

        if ci == 0:
            wpass(0, 21)
            eng.tensor_scalar_mul(q23[:, 0:20, :], zw3[:, 0:20, :], 0.75)
            eng.tensor_scalar_mul(r23[:, 0:21, :], zw3[:, 0:21, :], 0.25)
            eng.tensor_tensor(
                zu3[:, 2:39:2, :], r23[:, 0:19, :], q23[:, 1:20, :], Alu.add
            )
            eng.tensor_tensor(
                zu3[:, 1:40:2, :], r23[:, 1:21, :], q23[:, 0:20, :], Alu.add
            )
            eng.tensor_tensor(zu3[:, 0:1, :], r23[:, 0:1, :], q23[:, 0:1, :], Alu.add)
        else:
            wpass(21, 40)
            eng.tensor_scalar_mul(q23[:, 20:40, :], zw3[:, 20:40, :], 0.75)
            eng.tensor_scalar_mul(r23[:, 21:40, :], zw3[:, 21:40, :], 0.25)
            eng.tensor_tensor(
                zu3[:, 40:79:2, :], r23[:, 19:39, :], q23[:, 20:40, :], Alu.add
            )
            eng.tensor_tensor(
                zu3[:, 41:78:2, :], r23[:, 21:40, :], q23[:, 20:39, :], Alu.add
            )
            eng.tensor_tensor(
                zu3[:, 79:80, :], r23[:, 39:40, :], q23[:, 39:40, :], Alu.add
            )

    def row_rep(ap, np_, fd, n=64):
        # [np_, fd] -> [np_, n, fd] with a 0-step repeat free dim
        return ap.unsqueeze(1).broadcast_to((np_, n, fd))

    with ExitStack() as ctx:
        tc = ctx.enter_context(tile.TileContext(nc))
        wpool = ctx.enter_context(tc.tile_pool(name="w", bufs=1))
        ppool = ctx.enter_context(tc.tile_pool(name="pp", bufs=1))
        zzl_pool = ctx.enter_context(tc.tile_pool(name="zzl", bufs=2))
        q_pool = ctx.enter_context(tc.tile_pool(name="q", bufs=1))
        zw_pool = ctx.enter_context(tc.tile_pool(name="zw", bufs=1))
        q2_pool = ctx.enter_context(tc.tile_pool(name="q2", bufs=1))
        zzu_pool = ctx.enter_context(tc.tile_pool(name="zzu", bufs=2))
        r01_pool = ctx.enter_context(tc.tile_pool(name="r01", bufs=4))
        u0_pool = ctx.enter_context(tc.tile_pool(name="u0", bufs=4))
        u1_pool = ctx.enter_context(tc.tile_pool(name="u1", bufs=4))
        psum_pool = ctx.enter_context(tc.tile_pool(name="ps", bufs=4, space="PSUM"))
        feat_pool = ctx.enter_context(tc.tile_pool(name="f", bufs=4))
        gb_pool = ctx.enter_context(tc.tile_pool(name="gb", bufs=6))
        o_pool = ctx.enter_context(tc.tile_pool(name="o", bufs=4))

        U0 = wpool.tile([128, 512], bf16)
        U1 = wpool.tile([68, 512], bf16)
        nc.sync.dma_start(U0[:], u0w_h.ap()[:, :])
        nc.sync.dma_start(U1[:], u1w_h.ap()[:, :])

        # --- P pipe (once per core): partition-packed 64-wide, h on partitions.
        # Partition (b, row, octet o) holds 7 halo-clamped low rows; output is
        # that octet's 10 hi rows. Host pre-clamps, so no edge ops in H.
        Ppk = ppool.tile([64, 7 * WZ], bf16)
        nc.sync.dma_start(Ppk[:], pp8_h.ap()[:, :])
        Pqk = ppool.tile([64, 7 * WZ], bf16)
        Pwk = ppool.tile([64, 7 * WF], bf16)
        Pq2k = ppool.tile([64, 7 * WF], bf16)
        Puk = ppool.tile([64, 10 * WF], bf16)
        lo3 = Ppk[:].rearrange("p (h w) -> p h w", w=WZ)
        q3 = Pqk[:].rearrange("p (h w) -> p h w", w=WZ)
        w3 = Pwk[:].rearrange("p (h w) -> p h w", w=WF)
        q23 = Pq2k[:].rearrange("p (h w) -> p h w", w=WF)
        hi3 = Puk[:].rearrange("p (h w) -> p h w", w=WF)
        nc.vector.tensor_scalar_mul(Pqk[:], Ppk[:], 0.75)
        nc.vector.scalar_tensor_tensor(
            w3[:, :, 2::2], lo3[:, :, 0:39], 0.25, q3[:, :, 1:40], Alu.mult, Alu.add
        )
        nc.vector.scalar_tensor_tensor(
            w3[:, :, 1:79:2], lo3[:, :, 1:40], 0.25, q3[:, :, 0:39], Alu.mult, Alu.add
        )
        nc.vector.scalar_tensor_tensor(
            w3[:, :, 0:1], lo3[:, :, 0:1], 0.25, q3[:, :, 0:1], Alu.mult, Alu.add
        )
        nc.vector.scalar_tensor_tensor(
            w3[:, :, 79:80], lo3[:, :, 39:40], 0.25, q3[:, :, 39:40], Alu.mult, Alu.add
        )
        nc.vector.tensor_scalar_mul(Pq2k[:], Pwk[:], 0.75)
        nc.vector.scalar_tensor_tensor(
            hi3[:, 0:10:2, :], w3[:, 0:5, :], 0.25, q23[:, 1:6, :], Alu.mult, Alu.add
        )
        nc.vector.scalar_tensor_tensor(
            hi3[:, 1:10:2, :], w3[:, 2:7, :], 0.25, q23[:, 1:6, :], Alu.mult, Alu.add
        )
        # No unpack: groups are octet-aligned (GN=800), so broadcasts read
        # Puk directly via contiguous partition slices.

        # --- Z pipes: z duplicated into both partition halves, 128-wide ---
        zzu_tiles = {}

        def emit_zz_dma(b):
            ZZl = zzl_pool.tile([128, NLOW], bf16, name=f"ZZl{b}")
            nc.sync.dma_start(
                ZZl[:], zp_h.ap()[b].unsqueeze(0).broadcast_to((2, D, NLOW))
            )
            Zq = q_pool.tile([128, NLOW], bf16, name=f"Zq{b}", tag="Zq")
            Zw = zw_pool.tile([128, HZ * WF], bf16, name=f"Zw{b}", tag="Zw")
            Zq2 = q2_pool.tile([128, HZ * WF], bf16, name=f"Zq2{b}", tag="Zq2")
            ZZu = zzu_pool.tile([128, NPIX], bf16, name=f"ZZu{b}")
            zzu_tiles[b] = (ZZl, Zq, Zw, Zq2, ZZu)

        def emit_zz_chunk(b, ci):
            ZZl, Zq, Zw, Zq2, ZZu = zzu_tiles[b]
            upsample_chunk(nc.vector, ZZl[:], Zq[:], Zw[:], Zq2[:], ZZu[:], Alu, ci)

        u_tiles = {}
        pending_outs = []
        pair_tiles = {}

        def emit_ubuild(b, gi):
            ZZu = zzu_tiles[b][4]
            gs, gn = _groups()[gi]
            # packed-P partitions for this group's octet: (b, octet gi, row r)
            pbase = b * 32 + gi * 4
            # router replication + u build for this group's columns
            R01 = r01_pool.tile([128, gn], bf16)
            nc.sync.dma_start(
                R01[:], row_rep(Puk[pbase : pbase + 2, :], 2, gn)
            )
            u0 = u0_pool.tile([128, gn], bf16)
            u1 = u1_pool.tile([68, gn], bf16)
            # stage broadcast P_up[2] into u0's lower half, consume it for
            # u1, then overwrite u0 (same-engine WAR)
            nc.sync.dma_start(
                u0[0:64, :], row_rep(Puk[pbase + 2 : pbase + 3, :], 1, gn)
            )
            nc.vector.tensor_tensor(
                u1[0:64, :], ZZu[0:64, gs : gs + gn], u0[0:64, :], Alu.mult
            )
            nc.sync.dma_start(u1[64:68, :], Puk[pbase : pbase + 4, :])
            nc.vector.tensor_tensor(u0[:], ZZu[:, gs : gs + gn], R01[:], Alu.mult)
            u_tiles[(b, gi)] = (u0, u1)

        def emit_group(b, gi):
            featb = feat_h.ap()[b]
            outb = out_h.ap()[b]
            gs, gn = _groups()[gi]
            u0, u1 = u_tiles.pop((b, gi))
            if True:
                # one feat load / out store per PAIR of groups, both channel
                # halves: tile layout [h0(2gn) | h1(2gn)] on partitions 0..127
                if gi % 2 == 0:
                    pgs = gs
                    fdram = featb[:, pgs : pgs + 2 * gn].rearrange(
                        "(t c) x -> c t x", t=2
                    )
                    f2 = feat_pool.tile([128, 4 * gn], bf16)
                    nc.sync.dma_start(
                        f2[:].rearrange("p (t x) -> p t x", t=2), fdram
                    )
                    o2 = o_pool.tile([128, 4 * gn], bf16)
                    pair_tiles[b] = (f2, o2, pgs)
                f2, o2, pgs = pair_tiles[b]
                goff = gs - pgs  # 0 or gn within the pair span
                for sub in range(0, gn, FN):
                    sn = min(FN, gn - sub)
                    for half in range(2):
                        # [gamma(sn) | pad | beta(sn)]: beta at bank boundary
                        ps = psum_pool.tile([128, 1024], f32)
                        for ci, wo in ((half, 0), (2 + half, 512)):
                            nc.tensor.matmul(
                                ps[:, wo : wo + sn],
                                U0[:, ci * 128 : ci * 128 + 128],
                                u0[:, sub : sub + sn],
                                start=True, stop=False,
                            )
                            nc.tensor.matmul(
                                ps[:, wo : wo + sn],
                                U1[:, ci * 128 : ci * 128 + 128],
                                u1[:, sub : sub + sn],
                                start=False, stop=True,
                            )
                        gbc = gb_pool.tile([128, 2 * sn], bf16)
                        src = ps[:].rearrange("p (t x) -> p t x", x=512)  # [128,2,512]
                        nc.scalar.copy(
                            gbc[:].rearrange("p (t x) -> p t x", x=sn),
                            src[:, :, 0:sn],
                        )
                        co = half * 2 * gn + goff + sub
                        oslc = o2[:, co : co + sn]
                        fslc = f2[:, co : co + sn]
                        eng = nc.vector if half == 0 else nc.gpsimd
                        eng.tensor_tensor(oslc, fslc, gbc[:, 0:sn], Alu.mult)
                        nc.gpsimd.tensor_tensor(
                            oslc, oslc, gbc[:, sn : 2 * sn], Alu.add
                        )
                if gi % 2 == 1:
                    odram = outb[:, pgs : pgs + 2 * gn].rearrange(
                        "(t c) x -> c t x", t=2
                    )
                    pending_outs.append(
                        (odram, o2[:].rearrange("p (t x) -> p t x", t=2))
                    )

        def flush_out():
            # emit the oldest deferred out store; by now its Pool add is done,
            # so it doesn't head-of-line-block the SP queue
            if pending_outs:
                odram, osrc = pending_outs.pop(0)
                nc.sync.dma_start(odram, osrc)

        # --- interleaved, software-pipelined schedule: u-build (A) runs one
        # group ahead of matmul+FiLM (B) so the DVE queue never blocks on the
        # psum->FiLM chain; ZZ pipe chunks slot between groups.
        emit_zz_dma(0)
        emit_zz_chunk(0, 0)
        emit_ubuild(0, 0)
        emit_ubuild(0, 1)
        flush_out()
        emit_group(0, 0)
        emit_ubuild(0, 2)
        flush_out()
        emit_group(0, 1)
        emit_zz_chunk(0, 1)
        emit_ubuild(0, 3)
        flush_out()
        emit_group(0, 2)
        emit_ubuild(0, 4)
        flush_out()
        emit_group(0, 3)
        emit_zz_dma(1)
        emit_zz_chunk(1, 0)
        emit_ubuild(0, 5)
        flush_out()
        emit_group(0, 4)
        emit_ubuild(0, 6)
        flush_out()
        emit_group(0, 5)
        emit_ubuild(0, 7)
        flush_out()
        emit_group(0, 6)
        emit_ubuild(1, 0)
        flush_out()
        emit_group(0, 7)
        emit_ubuild(1, 1)
        flush_out()
        emit_group(1, 0)
        emit_zz_chunk(1, 1)
        emit_ubuild(1, 2)
        flush_out()
        emit_group(1, 1)
        emit_ubuild(1, 3)
        flush_out()
        emit_group(1, 2)
        emit_ubuild(1, 4)
        flush_out()
        emit_group(1, 3)
        emit_ubuild(1, 5)
        flush_out()
        emit_group(1, 4)
        emit_ubuild(1, 6)
        flush_out()
        emit_group(1, 5)
        emit_ubuild(1, 7)
        flush_out()
        emit_group(1, 6)
        flush_out()
        emit_group(1, 7)
        flush_out()
        flush_out()
    nc.compile()
    return nc


def _get_program():
    if "nc" not in _cache:
        _cache["nc"] = _build_program()
    return _cache["nc"]


def _pack_p(P):
    """[B, K, HZ, WZ] -> per-core [64, 7*WZ]: partition (b, row, octet) holds
    7 halo-clamped low rows (rows: p0,p1,p2,ones)."""
    plow = np.empty((B, 4, HZ, WZ), np.float32)
    plow[:, :K] = P.reshape(B, K, HZ, WZ)
    plow[:, K] = 1.0
    pp = np.empty((B, 4, 8, 7, WZ), np.float32)
    for o in range(8):
        idx = np.clip(np.arange(5 * o - 1, 5 * o + 6), 0, HZ - 1)
        pp[:, :, o] = plow[:, :, idx, :]
    pp = pp.transpose(0, 2, 1, 3, 4)  # (b, octet, row, 7, WZ)
    pp = np.ascontiguousarray(pp).astype(BF16).reshape(NCORES, BPC * 4 * 8, 7 * WZ)
    return [np.ascontiguousarray(pp[c]) for c in range(NCORES)]


def _prep_weights(Wg, bg, Wb, bb):
    U = np.zeros((196, 512), np.float32)
    for k in range(3):
        U[64 * k : 64 * (k + 1), 0:256] = Wg[k].T
        U[64 * k : 64 * (k + 1), 256:512] = Wb[k].T
    U[192:195, 0:256] = bg
    U[192:195, 256:512] = bb
    U[195, 0:256] = 1.0
    U[195, 256:512] = 0.0
    Ub = U.astype(BF16)
    return np.ascontiguousarray(Ub[0:128]), np.ascontiguousarray(Ub[128:196])


def kernel(**inputs):
    import concourse.bass_utils as bass_utils

    feat = np.asarray(inputs["feat"], dtype=np.float32)
    Z = np.asarray(inputs["Z"], dtype=np.float32)
    P = np.asarray(inputs["P"], dtype=np.float32)
    U0np, U1np = _prep_weights(
        np.asarray(inputs["Wg"], dtype=np.float32),
        np.asarray(inputs["bg"], dtype=np.float32),
        np.asarray(inputs["Wb"], dtype=np.float32),
        np.asarray(inputs["bb"], dtype=np.float32),
    )

    featb = feat.reshape(B, C, NPIX).astype(BF16)
    zpb = Z.reshape(B, D, NLOW).astype(BF16)
    pp8b = _pack_p(P)

    nc = _get_program()
    in_maps = []
    for c in range(NCORES):
        sl = slice(c * BPC, (c + 1) * BPC)
        in_maps.append(
            {
                "feat": np.ascontiguousarray(featb[sl]),
                "zp": np.ascontiguousarray(zpb[sl]),
                "pp8": pp8b[c],
                "u0w": U0np,
                "u1w": U1np,
            }
        )

    res = bass_utils.run_bass_kernel_spmd(nc, in_maps, core_ids=list(range(NCORES)))
    out = np.concatenate([r["out"] for r in res.results], axis=0)
    return out.astype(np.float32).reshape(B, C, HF, WF)


if __name__ == "__main__":
    import reference

    inputs = {k: np.asarray(v) for k, v in reference.setup_inputs().items()}
    out = kernel(**inputs)
    print("out", out.shape, out.dtype)
